# revision 22
# baseline (speedup 1.0000x reference)
"""APPNP (K=3, alpha=0.5) on 8 Trainium2 NeuronCores.

Distribution: 1D node partition (12500 dst-nodes per core), weights
replicated. Per propagation step each core re-gathers the full h-tilde
table (AllGather) and processes the ~400k edges whose dst it owns.

Per-core per-step pipeline (edges pre-sorted by dst on host, grouped by
src-chunk into 8 GPSIMD groups x 8 node-eighths):
  ap_gather (Q7, 8-way)  : msgs[g,ch,i] = table[src]        (f32)
  tensor_tensor_scan(DVE): P = inclusive prefix sum of msgs (f32, in-place)
  ap_gather (Q7)         : EN[j] = P[last-edge-slot(node j)]
  subtract (DVE)         : per-node segment sums (diff of adjacent ends)
  matmul (PE)            : sum the 8 src-group partials into packed layout
  axpby (DVE)            : h_next = w1*agg + nh0 ; AllGather -> table
"""
import numpy as np

# ---------------------------------------------------------------- config
NCORES = 8
CCH = 16          # channels
FEAT = 512        # input features
K_STEPS = 3
ALPHA = 0.5

# real-problem geometry (overridable for small-scale tests)
N_NODES = 100000
NPC = 12500       # nodes per core
SB = 1568         # nodes per eighth (sub-block); NPC_PAD = 8*SB
WIN = SB // 4     # PE window (<=512, one PSUM bank)


def _geom():
    npc_pad = 8 * SB
    assert WIN * 4 == SB and WIN <= 512
    assert NPC <= npc_pad <= 32768 - 1
    assert SB % 32 == 0  # SBC even: int16 idx slices stay 4-byte aligned
    return npc_pad


# ---------------------------------------------------------------- host prep
def _wrap16(arr):
    """[L] -> wrapped [16, L/16] layout (element i at [i%16, i//16])."""
    L = arr.shape[-1]
    assert L % 16 == 0
    return arr.reshape(L // 16, 16).T


def _pack_nodevec(v, npc_pad):
    """per-node vector [<=NPC] -> packed [128, SB]: row 16k+ch = sub-block
    k's values (replicated over ch); padded nodes -> 0."""
    vp = np.zeros(npc_pad, np.float32)
    vp[: v.shape[0]] = v
    blocks = vp.reshape(8, SB)
    out = np.zeros((128, SB), np.float32)
    for k in range(8):
        out[16 * k:16 * k + 16, :] = blocks[k][None, :]
    return out


def _to_bf16(a):
    import ml_dtypes
    return np.asarray(a).astype(ml_dtypes.bfloat16)


def _balance_eighths(dl, gl, rng_seed=0):
    """Assign this core's dst nodes to the 8 eighths so per-(group, eighth)
    edge counts are balanced (shrinks the padded stream length NH).
    dl/gl: per-edge local dst + src group. Returns perm[node] -> position."""
    deg = np.zeros((NPC, 8), np.int64)
    np.add.at(deg, (dl, gl), 1)
    tot = deg.sum(axis=1)
    order = np.argsort(-tot, kind="stable")
    loads = np.zeros((8, 8), np.int64)
    cnt = np.zeros(8, np.int64)
    cap = np.full(8, SB, np.int64)
    cap[7] = SB - 1          # keep the last slot of eighth 7 a zero pad
    perm = np.zeros(NPC, np.int64)
    for nid in order:
        d = deg[nid]
        cand = loads + d[None, :]
        score = cand.max(axis=1)
        score[cnt >= cap] = 1 << 60
        k = int(np.argmin(score))
        loads[k] += d
        perm[nid] = k * SB + cnt[k]
        cnt[k] += 1
    return perm


def prepare_inputs(x, W, b, src, dst):
    """Build per-core in_maps + global padded stream length NH."""
    npc_pad = _geom()
    n = x.shape[0]
    assert n == NCORES * NPC

    src = np.asarray(src, dtype=np.int64)
    dst = np.asarray(dst, dtype=np.int64)
    deg_out = np.bincount(src, minlength=n).astype(np.float32)
    deg_in = np.bincount(dst, minlength=n).astype(np.float32)
    norm_out = np.clip(deg_out, 1.0, None) ** -0.5
    norm_in = np.clip(deg_in, 1.0, None) ** -0.5

    owner = dst // NPC
    group = src // NPC
    srcloc = (src - group * NPC).astype(np.int32)
    dstloc = (dst - owner * NPC).astype(np.int32)

    # balanced node -> packed-position permutation per core
    perms = []
    pos_of = np.zeros_like(dstloc)
    for c in range(NCORES):
        m = owner == c
        perm = _balance_eighths(dstloc[m], group[m])
        perms.append(perm)
        pos_of[m] = perm[dstloc[m]].astype(np.int32)
    # gather indices address the src core's table, which lives in that
    # core's permuted position space
    pos_src = np.zeros_like(srcloc)
    for g in range(NCORES):
        m = group == g
        pos_src[m] = perms[g][srcloc[m]].astype(np.int32)
    srcloc = pos_src

    cell = (owner * 8 + group).astype(np.int64)
    order = np.argsort(cell * npc_pad + pos_of, kind="stable")
    srcloc_s = srcloc[order]
    dstloc_s = pos_of[order]
    cell_s = cell[order]
    cell_start = np.searchsorted(cell_s, np.arange(65), side="left")

    # pass 1: max eighth-stream length
    nh_max = 0
    cells = {}
    for c in range(NCORES):
        for g in range(8):
            lo, hi = cell_start[c * 8 + g], cell_start[c * 8 + g + 1]
            dl = dstloc_s[lo:hi]
            sl = srcloc_s[lo:hi]
            kb = np.searchsorted(dl, np.arange(0, npc_pad + 1, SB), side="left")
            cells[(c, g)] = (dl, sl, kb)
            nh_max = max(nh_max, int(np.max(kb[1:] - kb[:-1])))
    NH = ((nh_max + 1 + 31) // 32) * 32  # +1 sentinel; %32 keeps every
    # int16 idx slice offset 4-byte aligned (Q7 reads idx by words)
    assert NH <= 32767, f"stream too long: {NH}"
    NHC = NH // 16
    SBC = SB // 16
    SENT = np.int16(npc_pad - 1)                  # table col holding 0.0
    # (eighth 7 is capped at SB-1 real nodes, so the last slot is a pad)

    wt = np.ascontiguousarray(W.T).astype(np.float32)      # [FEAT, 16]
    wtA = np.zeros((FEAT, 32), np.float32)
    wtB = np.zeros((FEAT, 32), np.float32)
    wtA[:, :16] = wt
    wtB[:, 16:] = wt
    # pre-arranged for SBUF: [128, KC*32], chunk kc at cols 32kc..
    KC = FEAT // 128
    wtA = wtA.reshape(KC, 128, 32).transpose(1, 0, 2).reshape(128, KC * 32)
    wtB = wtB.reshape(KC, 128, 32).transpose(1, 0, 2).reshape(128, KC * 32)
    g32A = np.zeros((128, 32), np.float32)
    g32B = np.zeros((128, 32), np.float32)
    for p in range(128):
        g32A[p, p % 16] = 1.0
        g32B[p, 16 + p % 16] = 1.0
    bh32 = np.tile((ALPHA * b).astype(np.float32), 2).reshape(32, 1)

    in_maps = []
    for c in range(NCORES):
        gidx = np.full((128, 8 * NHC), SENT, np.int16)
        bidx = np.zeros((128, 8 * SBC), np.int16)
        for g in range(8):
            dl, sl, kb = cells[(c, g)]
            for k in range(8):
                lo, hi = kb[k], kb[k + 1]
                stream = np.full(NH, SENT, np.int16)
                stream[1:1 + hi - lo] = sl[lo:hi].astype(np.int16)
                gidx[16 * g:16 * g + 16, k * NHC:(k + 1) * NHC] = _wrap16(stream)
                ends = np.searchsorted(
                    dl[lo:hi], np.arange(SB * k, SB * (k + 1)), side="right"
                ).astype(np.int16)
                bidx[16 * g:16 * g + 16, k * SBC:(k + 1) * SBC] = _wrap16(ends)

        rows = slice(c * NPC, (c + 1) * NPC)
        perm = perms[c]
        xt = np.zeros((FEAT, npc_pad), np.float32)
        xt[:, perm] = x[rows].T
        no_c = np.zeros(npc_pad, np.float32)
        ni_c = np.zeros(npc_pad, np.float32)
        no_c[perm] = norm_out[rows]
        ni_c[perm] = norm_in[rows]
        in_maps.append({
            "xt": _to_bf16(xt),
            "wtA": _to_bf16(wtA),
            "wtB": _to_bf16(wtB),
            "g32A": g32A,
            "g32B": g32B,
            "bh32": bh32,
            "normoutp": _pack_nodevec(no_c, npc_pad),
            "w1p": _pack_nodevec((1 - ALPHA) * ni_c * no_c, npc_pad),
            "w2p": _pack_nodevec((1 - ALPHA) * ni_c, npc_pad),
            "gidx": gidx,
            "bidx": bidx,
        })
    return in_maps, NH, perms


# ---------------------------------------------------------------- device build
def build(NH):
    import concourse.mybir as mybir
    from concourse import bacc
    import concourse.tile as tile

    F32 = mybir.dt.float32
    BF16 = mybir.dt.bfloat16
    I16 = mybir.dt.int16
    npc_pad = _geom()
    NHC = NH // 16
    SBC = SB // 16
    KC = FEAT // 128

    nc = bacc.Bacc(None, target_bir_lowering=False, num_devices=NCORES)
    xt_e = nc.declare_dram_parameter("xt", [FEAT, npc_pad], BF16, isOutput=False)
    wtA_e = nc.declare_dram_parameter("wtA", [128, (FEAT // 128) * 32], BF16,
                                      isOutput=False)
    wtB_e = nc.declare_dram_parameter("wtB", [128, (FEAT // 128) * 32], BF16,
                                      isOutput=False)
    g32A_e = nc.declare_dram_parameter("g32A", [128, 32], F32, isOutput=False)
    g32B_e = nc.declare_dram_parameter("g32B", [128, 32], F32, isOutput=False)
    bh32_e = nc.declare_dram_parameter("bh32", [32, 1], F32, isOutput=False)
    nop_e = nc.declare_dram_parameter("normoutp", [128, SB], F32, isOutput=False)
    w1p_e = nc.declare_dram_parameter("w1p", [128, SB], F32, isOutput=False)
    w2p_e = nc.declare_dram_parameter("w2p", [128, SB], F32, isOutput=False)
    gidx_e = nc.declare_dram_parameter("gidx", [128, 8 * NHC], I16, isOutput=False)
    bidx_e = nc.declare_dram_parameter("bidx", [128, 8 * SBC], I16, isOutput=False)
    out_e = nc.declare_dram_parameter("out", [128, SB], F32, isOutput=True)
    import os
    DBG = bool(int(os.environ.get("KM_DEBUG", "0")))
    if DBG:
        dbg_tab_e = nc.declare_dram_parameter("dbg_tab", [128, npc_pad], F32,
                                              isOutput=True)
        dbg_p_e = nc.declare_dram_parameter("dbg_p", [128, NH], F32,
                                            isOutput=True)
        dbg_en_e = nc.declare_dram_parameter("dbg_en", [128, SB + 4], F32,
                                             isOutput=True)
        dbg_dif_e = nc.declare_dram_parameter("dbg_dif", [128, SB], F32,
                                              isOutput=True)

    ADD = mybir.AluOpType.add
    SUB = mybir.AluOpType.subtract
    MUL = mybir.AluOpType.mult
    BYP = mybir.AluOpType.bypass
    COPY = mybir.ActivationFunctionType.Copy

    with tile.TileContext(nc) as tc:
        with (
            tc.tile_pool(name="sbuf", bufs=1) as pool,
            tc.tile_pool(name="dram", bufs=1, space="DRAM") as dram,
        ):
            # --- persistent tiles
            table = pool.tile([128, npc_pad], F32, name="table")
            gidx = pool.tile([128, 8 * NHC], I16, name="gidx")
            bidx = pool.tile([128, 8 * SBC], I16, name="bidx")
            g32A = pool.tile([128, 32], F32, name="g32A")
            g32B = pool.tile([128, 32], F32, name="g32B")
            bh32 = pool.tile([32, 1], F32, name="bh32")
            ah0p = pool.tile([128, SB], F32, name="ah0p")
            nh0p = pool.tile([128, SB], F32, name="nh0p")
            nc.sync.dma_start(out=gidx[:], in_=gidx_e[:, :])
            nc.sync.dma_start(out=bidx[:], in_=bidx_e[:, :])
            nc.sync.dma_start(out=g32A[:], in_=g32A_e[:, :])
            nc.sync.dma_start(out=g32B[:], in_=g32B_e[:, :])
            nc.sync.dma_start(out=bh32[:], in_=bh32_e[:, :])

            # --- projection: packed ah0p = alpha*h0 (h0 = x @ W.T + b)
            wtA_t = pool.tile([128, KC * 32], BF16, name="wtA_t")
            wtB_t = pool.tile([128, KC * 32], BF16, name="wtB_t")
            nc.sync.dma_start(out=wtA_t[:], in_=wtA_e[:, :])
            nc.sync.dma_start(out=wtB_t[:], in_=wtB_e[:, :])

            with tc.tile_pool(name="projpsum", bufs=1, space="PSUM") as ppsum:
                for q in range(4):
                    pp = ppsum.tile([32, 4 * 512], F32, tag="projp", bufs=2,
                                    name=f"pp{q}")
                    xtq = {}
                    for kc in range(KC):
                        xt_t = pool.tile([128, 2 * SB], BF16, tag="xt",
                                         bufs=5, name=f"xt{q}{kc}")
                        eng = nc.sync if kc % 2 == 0 else nc.scalar
                        eng.dma_start(
                            out=xt_t[:],
                            in_=xt_e[128 * kc:128 * (kc + 1),
                                     2 * q * SB:2 * (q + 1) * SB])
                        xtq[kc] = xt_t
                    for w4 in range(4):
                        first = True
                        for e in range(2):
                            wt_t = wtA_t if e == 0 else wtB_t
                            base = SB * e + WIN * w4
                            for kc in range(KC):
                                nc.tensor.matmul(
                                    out=pp[:, 512 * w4:512 * w4 + WIN],
                                    lhsT=wt_t[:, 32 * kc:32 * (kc + 1)],
                                    rhs=xtq[kc][:, base:base + WIN],
                                    start=first,
                                    stop=(e == 1 and kc == KC - 1),
                                )
                                first = False
                    # ah0p[32q..] = ALPHA*psum + ALPHA*b
                    nc.vector.tensor_scalar(
                        out=ah0p[32 * q:32 * (q + 1), :].rearrange(
                            "p (w j) -> p w j", w=4),
                        in0=pp[:, :].rearrange(
                            "p (w j) -> p w j", w=4)[:, :, :WIN],
                        scalar1=ALPHA, scalar2=bh32[:, :],
                        op0=MUL, op1=ADD)

            normoutp = pool.tile([128, SB], F32, tag="wslot", bufs=1,
                                 name="normoutp")
            nc.sync.dma_start(out=normoutp[:], in_=nop_e[:, :])
            nc.vector.tensor_tensor(out=nh0p[:], in0=ah0p[:], in1=normoutp[:],
                                    op=MUL)

            agi = dram.tile([CCH, npc_pad], F32, name="agi")
            ago = dram.tile([128, npc_pad], F32, name="ago")

            def broadcast_table(hp_f32):
                """hp (packed f32) -> AllGather -> f32 table."""
                nc.sync.dma_start(
                    out=agi[:, :].rearrange("c (k j) -> k c j", k=8),
                    in_=hp_f32[:])
                nc.gpsimd.collective_compute(
                    "AllGather", BYP,
                    replica_groups=[list(range(NCORES))],
                    ins=[agi.opt()], outs=[ago.opt()])
                th = npc_pad // 3 // 32 * 32
                nc.sync.dma_start(out=table[:, :th], in_=ago[:, :th])
                nc.scalar.dma_start(out=table[:, th:2 * th],
                                    in_=ago[:, th:2 * th])
                nc.gpsimd.dma_start(out=table[:, 2 * th:],
                                    in_=ago[:, 2 * th:])

            # h~_0 = 2 * nh0p   (= norm_out * h0 for ALPHA=0.5)
            hp0 = pool.tile([128, SB], F32, tag="hp", bufs=2, name="hp0")
            nc.vector.tensor_scalar(out=hp0[:], in0=nh0p[:], scalar1=1.0 / ALPHA,
                                    scalar2=None, op0=MUL)
            broadcast_table(hp0)
            if DBG:
                nc.sync.dma_start(out=dbg_tab_e[:, :], in_=table[:])

            w1p = pool.tile([128, SB], F32, tag="wslot", bufs=1, name="w1p")
            nc.sync.dma_start(out=w1p[:], in_=w1p_e[:, :])

            # --- propagation steps
            with tc.tile_pool(name="aggpsum", bufs=1, space="PSUM") as apsum:
                for t in range(K_STEPS):
                    last = t == K_STEPS - 1
                    if last:
                        w2p = pool.tile([128, SB], F32, tag="wslot", bufs=1,
                                        name="w2p")
                        nc.sync.dma_start(out=w2p[:], in_=w2p_e[:, :])
                    ap = apsum.tile([128, 4 * 512], F32, tag="agg", bufs=2,
                                    name=f"agg{t}")
                    hp = pool.tile([128, SB], F32, tag="hp", bufs=2,
                                   name=f"hp{t + 1}")
                    pbl = {}
                    for k in range(9):
                        if k < 8:
                            msgs = pool.tile([128, NH], F32, tag="msgs",
                                             bufs=2, name=f"m{t}_{k}")
                            nc.gpsimd.ap_gather(
                                out_ap=msgs[:], in_ap=table[:],
                                idxs_ap=gidx[:, k * NHC:(k + 1) * NHC],
                                channels=128, num_elems=npc_pad, d=1,
                                num_idxs=NH)
                            # in-place inclusive prefix sum
                            import os as _os
                            _scan_eng = nc.gpsimd if _os.environ.get(
                                "KM_SCAN_POOL") else nc.vector
                            _scan_eng.tensor_tensor_scan(
                                out=msgs[:], data0=msgs[:], data1=msgs[:],
                                initial=0.0, op0=ADD, op1=BYP)
                            pbl[k] = msgs
                        if k >= 1:
                            kk = k - 1
                            en = pool.tile([128, SB + 4], F32, tag="en",
                                           bufs=2, name=f"en{t}_{kk}")
                            nc.gpsimd.ap_gather(
                                out_ap=en[:, 1:SB + 1], in_ap=pbl[kk][:],
                                idxs_ap=bidx[:, kk * SBC:(kk + 1) * SBC],
                                channels=128, num_elems=NH, d=1, num_idxs=SB)
                            nc.vector.memset(en[:, 0:1], 0)
                            dif = pool.tile([128, SB], F32, tag="dif", bufs=2,
                                            name=f"d{t}_{kk}")
                            nc.vector.tensor_tensor(
                                out=dif[:], in0=en[:, 1:SB + 1],
                                in1=en[:, 0:SB], op=SUB)
                            if DBG and t == 0 and kk == int(os.environ.get("KM_DBG_KK", "0")):
                                nc.sync.dma_start(out=dbg_p_e[:, :],
                                                  in_=pbl[kk][:])
                                nc.sync.dma_start(out=dbg_en_e[:, :],
                                                  in_=en[:])
                                nc.sync.dma_start(out=dbg_dif_e[:, :],
                                                  in_=dif[:])
                            e, q = kk % 2, kk // 2
                            for w in range(4):
                                nc.tensor.matmul(
                                    out=ap[32 * q:32 * (q + 1),
                                           512 * w:512 * w + WIN],
                                    lhsT=(g32A if e == 0 else g32B)[:, :],
                                    rhs=dif[:, WIN * w:WIN * (w + 1)],
                                    start=(e == 0), stop=(e == 1),
                                    tile_position=(0, 32 * q))
                            # after eighth 5: rows 0:96 final -> update +
                            # AllGather chunk A early so it overlaps the
                            # remaining eighths; chunk B (rows 96:128) +
                            # table fills land at the step boundary.
                            chunk = None
                            if kk == 5:
                                chunk = (0, 96)
                            elif kk == 7:
                                chunk = (96, 128)
                            if chunk is not None:
                                r0, r1 = chunk
                                wsel = w2p if last else w1p
                                asel = ah0p if last else nh0p
                                nc.vector.tensor_tensor(
                                    out=hp[r0:r1, :].rearrange(
                                        "p (w j) -> p w j", w=4),
                                    in0=ap[r0:r1, :].rearrange(
                                        "p (w j) -> p w j", w=4)[:, :, :WIN],
                                    in1=wsel[r0:r1, :].rearrange(
                                        "p (w j) -> p w j", w=4),
                                    op=MUL)
                                nc.vector.tensor_tensor(
                                    out=hp[r0:r1, :], in0=hp[r0:r1, :],
                                    in1=asel[r0:r1, :], op=ADD)
                                if not last:
                                    nk = (r1 - r0) // 16
                                    agiq = dram.tile(
                                        [CCH, nk * SB], F32, tag=f"agi{r0}",
                                        bufs=2, name=f"agi{t}_{r0}")
                                    agoq = dram.tile(
                                        [128, nk * SB], F32, tag=f"ago{r0}",
                                        bufs=2, name=f"ago{t}_{r0}")
                                    nc.sync.dma_start(
                                        out=agiq[:, :].rearrange(
                                            "c (k j) -> k c j", k=nk),
                                        in_=hp[r0:r1, :])
                                    nc.gpsimd.collective_compute(
                                        "AllGather", BYP,
                                        replica_groups=[list(range(NCORES))],
                                        ins=[agiq.opt()], outs=[agoq.opt()])
                                    if kk == 7:
                                        # boundary fills: chunk A split over
                                        # three engines (its AllGather is
                                        # long done); chunk B over two
                                        cA = 6 * SB
                                        c3 = cA // 3 // 32 * 32
                                        nc.sync.dma_start(
                                            out=table[:, :c3],
                                            in_=agoqA[:, :c3])
                                        nc.scalar.dma_start(
                                            out=table[:, c3:2 * c3],
                                            in_=agoqA[:, c3:2 * c3])
                                        nc.gpsimd.dma_start(
                                            out=table[:, 2 * c3:cA],
                                            in_=agoqA[:, 2 * c3:])
                                        nc.sync.dma_start(
                                            out=table[:, cA:cA + SB],
                                            in_=agoq[:, :SB])
                                        nc.scalar.dma_start(
                                            out=table[:, cA + SB:],
                                            in_=agoq[:, SB:])
                                    else:
                                        agoqA = agoq
                    if last:
                        nc.sync.dma_start(out=out_e[:, :], in_=hp[:])
    nc.compile()
    return nc


# ---------------------------------------------------------------- entry point
LAST_EXEC_NS = None


def kernel(x, W, b, src, dst):
    import os
    global LAST_EXEC_NS
    x = np.asarray(x, np.float32)
    W = np.asarray(W, np.float32)
    b = np.asarray(b, np.float32)
    in_maps, NH, perms = prepare_inputs(x, W, b, src, dst)
    nc = build(NH)
    from concourse.bass_utils import run_bass_kernel_spmd
    trace = bool(int(os.environ.get("KM_TRACE", "0")))
    res = run_bass_kernel_spmd(nc, in_maps, core_ids=list(range(NCORES)),
                               trace=trace)
    LAST_EXEC_NS = res.exec_time_ns
    out = np.empty((NCORES * NPC, CCH), np.float32)
    for c in range(NCORES):
        oc = np.asarray(res.results[c]["out"], np.float32)   # [128, SB]
        arr = oc.reshape(8, CCH, SB).transpose(0, 2, 1).reshape(8 * SB, CCH)
        out[c * NPC:(c + 1) * NPC] = arr[perms[c]]
    return out



# revision 26
# speedup vs baseline: 1.0101x; 1.0101x over previous
"""APPNP (K=3, alpha=0.5) on 8 Trainium2 NeuronCores.

Distribution: 1D node partition (12500 dst-nodes per core), weights
replicated. Per propagation step each core re-gathers the full h-tilde
table (AllGather) and processes the ~400k edges whose dst it owns.

Per-core per-step pipeline (edges pre-sorted by dst on host, grouped by
src-chunk into 8 GPSIMD groups x 8 node-eighths):
  ap_gather (Q7, 8-way)  : msgs[g,ch,i] = table[src]        (f32)
  tensor_tensor_scan(DVE): P = inclusive prefix sum of msgs (f32, in-place)
  ap_gather (Q7)         : EN[j] = P[last-edge-slot(node j)]
  subtract (DVE)         : per-node segment sums (diff of adjacent ends)
  matmul (PE)            : sum the 8 src-group partials into packed layout
  axpby (DVE)            : h_next = w1*agg + nh0 ; AllGather -> table
"""
import numpy as np

# ---------------------------------------------------------------- config
NCORES = 8
CCH = 16          # channels
FEAT = 512        # input features
K_STEPS = 3
ALPHA = 0.5

# real-problem geometry (overridable for small-scale tests)
N_NODES = 100000
NPC = 12500       # nodes per core
SB = 1568         # nodes per eighth (sub-block); NPC_PAD = 8*SB
WIN = SB // 4     # PE window (<=512, one PSUM bank)


def _geom():
    npc_pad = 8 * SB
    assert WIN * 4 == SB and WIN <= 512
    assert NPC <= npc_pad <= 32768 - 1
    assert SB % 32 == 0  # SBC even: int16 idx slices stay 4-byte aligned
    return npc_pad


# ---------------------------------------------------------------- host prep
def _wrap16(arr):
    """[L] -> wrapped [16, L/16] layout (element i at [i%16, i//16])."""
    L = arr.shape[-1]
    assert L % 16 == 0
    return arr.reshape(L // 16, 16).T


def _pack_nodevec(v, npc_pad):
    """per-node vector [<=NPC] -> packed [128, SB]: row 16k+ch = sub-block
    k's values (replicated over ch); padded nodes -> 0."""
    vp = np.zeros(npc_pad, np.float32)
    vp[: v.shape[0]] = v
    blocks = vp.reshape(8, SB)
    out = np.zeros((128, SB), np.float32)
    for k in range(8):
        out[16 * k:16 * k + 16, :] = blocks[k][None, :]
    return out


def _to_bf16(a):
    import ml_dtypes
    return np.asarray(a).astype(ml_dtypes.bfloat16)


def _balance_eighths(dl, gl, rng_seed=0):
    """Assign this core's dst nodes to the 8 eighths so per-(group, eighth)
    edge counts are balanced (shrinks the padded stream length NH).
    dl/gl: per-edge local dst + src group. Returns perm[node] -> position."""
    deg = np.zeros((NPC, 8), np.int64)
    np.add.at(deg, (dl, gl), 1)
    tot = deg.sum(axis=1)
    order = np.argsort(-tot, kind="stable")
    loads = np.zeros((8, 8), np.int64)
    cnt = np.zeros(8, np.int64)
    cap = np.full(8, SB, np.int64)
    cap[7] = SB - 1          # keep the last slot of eighth 7 a zero pad
    perm = np.zeros(NPC, np.int64)
    for nid in order:
        d = deg[nid]
        cand = loads + d[None, :]
        score = cand.max(axis=1)
        score[cnt >= cap] = 1 << 60
        k = int(np.argmin(score))
        loads[k] += d
        perm[nid] = k * SB + cnt[k]
        cnt[k] += 1
    return perm


def prepare_inputs(x, W, b, src, dst):
    """Build per-core in_maps + global padded stream length NH."""
    npc_pad = _geom()
    n = x.shape[0]
    assert n == NCORES * NPC

    src = np.asarray(src, dtype=np.int64)
    dst = np.asarray(dst, dtype=np.int64)
    deg_out = np.bincount(src, minlength=n).astype(np.float32)
    deg_in = np.bincount(dst, minlength=n).astype(np.float32)
    norm_out = np.clip(deg_out, 1.0, None) ** -0.5
    norm_in = np.clip(deg_in, 1.0, None) ** -0.5

    owner = dst // NPC
    group = src // NPC
    srcloc = (src - group * NPC).astype(np.int32)
    dstloc = (dst - owner * NPC).astype(np.int32)

    # balanced node -> packed-position permutation per core
    perms = []
    pos_of = np.zeros_like(dstloc)
    for c in range(NCORES):
        m = owner == c
        perm = _balance_eighths(dstloc[m], group[m])
        perms.append(perm)
        pos_of[m] = perm[dstloc[m]].astype(np.int32)
    # gather indices address the src core's table, which lives in that
    # core's permuted position space
    pos_src = np.zeros_like(srcloc)
    for g in range(NCORES):
        m = group == g
        pos_src[m] = perms[g][srcloc[m]].astype(np.int32)
    srcloc = pos_src

    cell = (owner * 8 + group).astype(np.int64)
    order = np.argsort(cell * npc_pad + pos_of, kind="stable")
    srcloc_s = srcloc[order]
    dstloc_s = pos_of[order]
    cell_s = cell[order]
    cell_start = np.searchsorted(cell_s, np.arange(65), side="left")

    # pass 1: max eighth-stream length
    nh_max = 0
    cells = {}
    for c in range(NCORES):
        for g in range(8):
            lo, hi = cell_start[c * 8 + g], cell_start[c * 8 + g + 1]
            dl = dstloc_s[lo:hi]
            sl = srcloc_s[lo:hi]
            kb = np.searchsorted(dl, np.arange(0, npc_pad + 1, SB), side="left")
            cells[(c, g)] = (dl, sl, kb)
            nh_max = max(nh_max, int(np.max(kb[1:] - kb[:-1])))
    NH = ((nh_max + 1 + 31) // 32) * 32  # +1 sentinel; %32 keeps every
    # int16 idx slice offset 4-byte aligned (Q7 reads idx by words)
    assert NH <= 32767, f"stream too long: {NH}"
    NHC = NH // 16
    SBC = SB // 16
    SENT = np.int16(npc_pad - 1)                  # table col holding 0.0
    # (eighth 7 is capped at SB-1 real nodes, so the last slot is a pad)

    wt = np.ascontiguousarray(W.T).astype(np.float32)      # [FEAT, 16]
    wtA = np.zeros((FEAT, 32), np.float32)
    wtB = np.zeros((FEAT, 32), np.float32)
    wtA[:, :16] = wt
    wtB[:, 16:] = wt
    # pre-arranged for SBUF: [128, KC*32], chunk kc at cols 32kc..
    KC = FEAT // 128
    wtA = wtA.reshape(KC, 128, 32).transpose(1, 0, 2).reshape(128, KC * 32)
    wtB = wtB.reshape(KC, 128, 32).transpose(1, 0, 2).reshape(128, KC * 32)
    g32A = np.zeros((128, 32), np.float32)
    g32B = np.zeros((128, 32), np.float32)
    for p in range(128):
        g32A[p, p % 16] = 1.0
        g32B[p, 16 + p % 16] = 1.0
    bh32 = np.tile((ALPHA * b).astype(np.float32), 2).reshape(32, 1)

    in_maps = []
    for c in range(NCORES):
        gidx = np.full((128, 8 * NHC), SENT, np.int16)
        bidx = np.zeros((128, 8 * SBC), np.int16)
        for g in range(8):
            dl, sl, kb = cells[(c, g)]
            for k in range(8):
                lo, hi = kb[k], kb[k + 1]
                stream = np.full(NH, SENT, np.int16)
                stream[1:1 + hi - lo] = sl[lo:hi].astype(np.int16)
                gidx[16 * g:16 * g + 16, k * NHC:(k + 1) * NHC] = _wrap16(stream)
                ends = np.searchsorted(
                    dl[lo:hi], np.arange(SB * k, SB * (k + 1)), side="right"
                ).astype(np.int16)
                bidx[16 * g:16 * g + 16, k * SBC:(k + 1) * SBC] = _wrap16(ends)

        rows = slice(c * NPC, (c + 1) * NPC)
        perm = perms[c]
        xt = np.zeros((FEAT, npc_pad), np.float32)
        xt[:, perm] = x[rows].T
        no_c = np.zeros(npc_pad, np.float32)
        ni_c = np.zeros(npc_pad, np.float32)
        no_c[perm] = norm_out[rows]
        ni_c[perm] = norm_in[rows]
        in_maps.append({
            "xt": _to_bf16(xt),
            "wtA": _to_bf16(wtA),
            "wtB": _to_bf16(wtB),
            "g32A": g32A,
            "g32B": g32B,
            "bh32": bh32,
            "normoutp": _pack_nodevec(no_c, npc_pad),
            "w1p": _pack_nodevec((1 - ALPHA) * ni_c * no_c, npc_pad),
            "w2p": _pack_nodevec((1 - ALPHA) * ni_c, npc_pad),
            "gidx": gidx,
            "bidx": bidx,
        })
    return in_maps, NH, perms


# ---------------------------------------------------------------- device build
def build(NH):
    import concourse.mybir as mybir
    from concourse import bacc
    import concourse.tile as tile

    F32 = mybir.dt.float32
    BF16 = mybir.dt.bfloat16
    I16 = mybir.dt.int16
    npc_pad = _geom()
    NHC = NH // 16
    SBC = SB // 16
    KC = FEAT // 128

    nc = bacc.Bacc(None, target_bir_lowering=False, num_devices=NCORES)
    xt_e = nc.declare_dram_parameter("xt", [FEAT, npc_pad], BF16, isOutput=False)
    wtA_e = nc.declare_dram_parameter("wtA", [128, (FEAT // 128) * 32], BF16,
                                      isOutput=False)
    wtB_e = nc.declare_dram_parameter("wtB", [128, (FEAT // 128) * 32], BF16,
                                      isOutput=False)
    g32A_e = nc.declare_dram_parameter("g32A", [128, 32], F32, isOutput=False)
    g32B_e = nc.declare_dram_parameter("g32B", [128, 32], F32, isOutput=False)
    bh32_e = nc.declare_dram_parameter("bh32", [32, 1], F32, isOutput=False)
    nop_e = nc.declare_dram_parameter("normoutp", [128, SB], F32, isOutput=False)
    w1p_e = nc.declare_dram_parameter("w1p", [128, SB], F32, isOutput=False)
    w2p_e = nc.declare_dram_parameter("w2p", [128, SB], F32, isOutput=False)
    gidx_e = nc.declare_dram_parameter("gidx", [128, 8 * NHC], I16, isOutput=False)
    bidx_e = nc.declare_dram_parameter("bidx", [128, 8 * SBC], I16, isOutput=False)
    out_e = nc.declare_dram_parameter("out", [128, SB], F32, isOutput=True)
    import os
    DBG = bool(int(os.environ.get("KM_DEBUG", "0")))
    if DBG:
        dbg_tab_e = nc.declare_dram_parameter("dbg_tab", [128, npc_pad], F32,
                                              isOutput=True)
        dbg_p_e = nc.declare_dram_parameter("dbg_p", [128, NH], F32,
                                            isOutput=True)
        dbg_en_e = nc.declare_dram_parameter("dbg_en", [128, SB + 4], F32,
                                             isOutput=True)
        dbg_dif_e = nc.declare_dram_parameter("dbg_dif", [128, SB], F32,
                                              isOutput=True)

    ADD = mybir.AluOpType.add
    SUB = mybir.AluOpType.subtract
    MUL = mybir.AluOpType.mult
    BYP = mybir.AluOpType.bypass
    COPY = mybir.ActivationFunctionType.Copy

    with tile.TileContext(nc) as tc:
        with (
            tc.tile_pool(name="sbuf", bufs=1) as pool,
            tc.tile_pool(name="dram", bufs=1, space="DRAM") as dram,
        ):
            # --- persistent tiles
            table = pool.tile([128, npc_pad], F32, name="table")
            gidx = pool.tile([128, 8 * NHC], I16, name="gidx")
            bidx = pool.tile([128, 8 * SBC], I16, name="bidx")
            g32A = pool.tile([128, 32], F32, name="g32A")
            g32B = pool.tile([128, 32], F32, name="g32B")
            bh32 = pool.tile([32, 1], F32, name="bh32")
            ah0p = pool.tile([128, SB], F32, name="ah0p")
            nh0p = pool.tile([128, SB], F32, name="nh0p")
            nc.sync.dma_start(out=gidx[:], in_=gidx_e[:, :])
            nc.sync.dma_start(out=bidx[:], in_=bidx_e[:, :])
            nc.sync.dma_start(out=g32A[:], in_=g32A_e[:, :])
            nc.sync.dma_start(out=g32B[:], in_=g32B_e[:, :])
            nc.sync.dma_start(out=bh32[:], in_=bh32_e[:, :])

            # --- projection: packed ah0p = alpha*h0 (h0 = x @ W.T + b)
            wtA_t = pool.tile([128, KC * 32], BF16, name="wtA_t")
            wtB_t = pool.tile([128, KC * 32], BF16, name="wtB_t")
            nc.sync.dma_start(out=wtA_t[:], in_=wtA_e[:, :])
            nc.sync.dma_start(out=wtB_t[:], in_=wtB_e[:, :])

            normoutp = pool.tile([128, SB], F32, tag="wslot", bufs=1,
                                 name="normoutp")
            nc.sync.dma_start(out=normoutp[:], in_=nop_e[:, :])
            # h~_0 = 2 * nh0p   (= norm_out * h0 for ALPHA=0.5)
            hp0 = pool.tile([128, SB], F32, tag="hp", bufs=2, name="hp0")
            agi = dram.tile([CCH, npc_pad], F32, name="agi")
            ago = dram.tile([128, npc_pad], F32, name="ago")

            with tc.tile_pool(name="projpsum", bufs=1, space="PSUM") as ppsum:
                for q in range(4):
                    pp = ppsum.tile([32, 4 * 512], F32, tag="projp", bufs=2,
                                    name=f"pp{q}")
                    xtq = {}
                    for kc in range(KC):
                        xt_t = pool.tile([128, 2 * SB], BF16, tag="xt",
                                         bufs=5, name=f"xt{q}{kc}")
                        eng = nc.sync if kc % 2 == 0 else nc.scalar
                        eng.dma_start(
                            out=xt_t[:],
                            in_=xt_e[128 * kc:128 * (kc + 1),
                                     2 * q * SB:2 * (q + 1) * SB])
                        xtq[kc] = xt_t
                    for w4 in range(4):
                        first = True
                        for e in range(2):
                            wt_t = wtA_t if e == 0 else wtB_t
                            base = SB * e + WIN * w4
                            for kc in range(KC):
                                nc.tensor.matmul(
                                    out=pp[:, 512 * w4:512 * w4 + WIN],
                                    lhsT=wt_t[:, 32 * kc:32 * (kc + 1)],
                                    rhs=xtq[kc][:, base:base + WIN],
                                    start=first,
                                    stop=(e == 1 and kc == KC - 1),
                                )
                                first = False
                    # ah0p[32q..] = ALPHA*psum + ALPHA*b
                    r0, r1 = 32 * q, 32 * (q + 1)
                    nc.vector.tensor_scalar(
                        out=ah0p[r0:r1, :].rearrange(
                            "p (w j) -> p w j", w=4),
                        in0=pp[:, :].rearrange(
                            "p (w j) -> p w j", w=4)[:, :, :WIN],
                        scalar1=ALPHA, scalar2=bh32[:, :],
                        op0=MUL, op1=ADD)
                    # finalize this quadrant of h~_0 and broadcast it now so
                    # the AllGather + table fill overlap the remaining
                    # projection quadrants
                    nc.vector.tensor_tensor(
                        out=nh0p[r0:r1, :], in0=ah0p[r0:r1, :],
                        in1=normoutp[r0:r1, :], op=MUL)
                    nc.vector.tensor_scalar(
                        out=hp0[r0:r1, :], in0=nh0p[r0:r1, :],
                        scalar1=1.0 / ALPHA, scalar2=None, op0=MUL)
                    agi0 = dram.tile([CCH, 2 * SB], F32, tag="agiP", bufs=4,
                                     name=f"agi0_{q}")
                    ago0 = dram.tile([128, 2 * SB], F32, tag="agoP", bufs=4,
                                     name=f"ago0_{q}")
                    nc.sync.dma_start(
                        out=agi0[:, :].rearrange("c (k j) -> k c j", k=2),
                        in_=hp0[r0:r1, :])
                    nc.gpsimd.collective_compute(
                        "AllGather", BYP,
                        replica_groups=[list(range(NCORES))],
                        ins=[agi0.opt()], outs=[ago0.opt()])
                    base = 2 * q * SB
                    nc.sync.dma_start(out=table[:, base:base + SB],
                                      in_=ago0[:, :SB])
                    nc.scalar.dma_start(out=table[:, base + SB:base + 2 * SB],
                                        in_=ago0[:, SB:])

            if DBG:
                nc.sync.dma_start(out=dbg_tab_e[:, :], in_=table[:])

            w1p = pool.tile([128, SB], F32, tag="wslot", bufs=1, name="w1p")
            nc.sync.dma_start(out=w1p[:], in_=w1p_e[:, :])

            # --- propagation steps
            with tc.tile_pool(name="aggpsum", bufs=1, space="PSUM") as apsum:
                for t in range(K_STEPS):
                    last = t == K_STEPS - 1
                    if last:
                        w2p = pool.tile([128, SB], F32, tag="wslot", bufs=1,
                                        name="w2p")
                        nc.sync.dma_start(out=w2p[:], in_=w2p_e[:, :])
                    ap = apsum.tile([128, 4 * 512], F32, tag="agg", bufs=2,
                                    name=f"agg{t}")
                    hp = pool.tile([128, SB], F32, tag="hp", bufs=2,
                                   name=f"hp{t + 1}")
                    pbl = {}
                    for k in range(9):
                        if k < 8:
                            msgs = pool.tile([128, NH], F32, tag="msgs",
                                             bufs=2, name=f"m{t}_{k}")
                            nc.gpsimd.ap_gather(
                                out_ap=msgs[:], in_ap=table[:],
                                idxs_ap=gidx[:, k * NHC:(k + 1) * NHC],
                                channels=128, num_elems=npc_pad, d=1,
                                num_idxs=NH)
                            # in-place inclusive prefix sum
                            import os as _os
                            _scan_eng = nc.gpsimd if _os.environ.get(
                                "KM_SCAN_POOL") else nc.vector
                            _scan_eng.tensor_tensor_scan(
                                out=msgs[:], data0=msgs[:], data1=msgs[:],
                                initial=0.0, op0=ADD, op1=BYP)
                            pbl[k] = msgs
                        if k >= 1:
                            kk = k - 1
                            en = pool.tile([128, SB + 4], F32, tag="en",
                                           bufs=2, name=f"en{t}_{kk}")
                            nc.gpsimd.ap_gather(
                                out_ap=en[:, 1:SB + 1], in_ap=pbl[kk][:],
                                idxs_ap=bidx[:, kk * SBC:(kk + 1) * SBC],
                                channels=128, num_elems=NH, d=1, num_idxs=SB)
                            nc.vector.memset(en[:, 0:1], 0)
                            dif = pool.tile([128, SB], F32, tag="dif", bufs=2,
                                            name=f"d{t}_{kk}")
                            nc.vector.tensor_tensor(
                                out=dif[:], in0=en[:, 1:SB + 1],
                                in1=en[:, 0:SB], op=SUB)
                            if DBG and t == 0 and kk == int(os.environ.get("KM_DBG_KK", "0")):
                                nc.sync.dma_start(out=dbg_p_e[:, :],
                                                  in_=pbl[kk][:])
                                nc.sync.dma_start(out=dbg_en_e[:, :],
                                                  in_=en[:])
                                nc.sync.dma_start(out=dbg_dif_e[:, :],
                                                  in_=dif[:])
                            e, q = kk % 2, kk // 2
                            for w in range(4):
                                nc.tensor.matmul(
                                    out=ap[32 * q:32 * (q + 1),
                                           512 * w:512 * w + WIN],
                                    lhsT=(g32A if e == 0 else g32B)[:, :],
                                    rhs=dif[:, WIN * w:WIN * (w + 1)],
                                    start=(e == 0), stop=(e == 1),
                                    tile_position=(0, 32 * q))
                            # after eighth 5: rows 0:96 final -> update +
                            # AllGather chunk A early so it overlaps the
                            # remaining eighths; chunk B (rows 96:128) +
                            # table fills land at the step boundary.
                            chunk = None
                            if kk == 5:
                                chunk = (0, 96)
                            elif kk == 7:
                                chunk = (96, 128)
                            if chunk is not None:
                                r0, r1 = chunk
                                wsel = w2p if last else w1p
                                asel = ah0p if last else nh0p
                                nc.vector.tensor_tensor(
                                    out=hp[r0:r1, :].rearrange(
                                        "p (w j) -> p w j", w=4),
                                    in0=ap[r0:r1, :].rearrange(
                                        "p (w j) -> p w j", w=4)[:, :, :WIN],
                                    in1=wsel[r0:r1, :].rearrange(
                                        "p (w j) -> p w j", w=4),
                                    op=MUL)
                                nc.vector.tensor_tensor(
                                    out=hp[r0:r1, :], in0=hp[r0:r1, :],
                                    in1=asel[r0:r1, :], op=ADD)
                                if not last:
                                    nk = (r1 - r0) // 16
                                    agiq = dram.tile(
                                        [CCH, nk * SB], F32, tag=f"agi{r0}",
                                        bufs=2, name=f"agi{t}_{r0}")
                                    agoq = dram.tile(
                                        [128, nk * SB], F32, tag=f"ago{r0}",
                                        bufs=2, name=f"ago{t}_{r0}")
                                    nc.sync.dma_start(
                                        out=agiq[:, :].rearrange(
                                            "c (k j) -> k c j", k=nk),
                                        in_=hp[r0:r1, :])
                                    nc.gpsimd.collective_compute(
                                        "AllGather", BYP,
                                        replica_groups=[list(range(NCORES))],
                                        ins=[agiq.opt()], outs=[agoq.opt()])
                                    if kk == 7:
                                        # boundary fills: both chunks, two
                                        # queues, interleaved halves
                                        cA = 6 * SB
                                        nc.sync.dma_start(
                                            out=table[:, :cA // 2],
                                            in_=agoqA[:, :cA // 2])
                                        nc.scalar.dma_start(
                                            out=table[:, cA // 2:cA],
                                            in_=agoqA[:, cA // 2:])
                                        nc.sync.dma_start(
                                            out=table[:, cA:cA + SB],
                                            in_=agoq[:, :SB])
                                        nc.scalar.dma_start(
                                            out=table[:, cA + SB:],
                                            in_=agoq[:, SB:])
                                    else:
                                        agoqA = agoq
                    if last:
                        nc.sync.dma_start(out=out_e[:, :], in_=hp[:])
    nc.compile()
    return nc


# ---------------------------------------------------------------- entry point
LAST_EXEC_NS = None


def kernel(x, W, b, src, dst):
    import os
    global LAST_EXEC_NS
    x = np.asarray(x, np.float32)
    W = np.asarray(W, np.float32)
    b = np.asarray(b, np.float32)
    in_maps, NH, perms = prepare_inputs(x, W, b, src, dst)
    nc = build(NH)
    from concourse.bass_utils import run_bass_kernel_spmd
    trace = bool(int(os.environ.get("KM_TRACE", "0")))
    res = run_bass_kernel_spmd(nc, in_maps, core_ids=list(range(NCORES)),
                               trace=trace)
    LAST_EXEC_NS = res.exec_time_ns
    out = np.empty((NCORES * NPC, CCH), np.float32)
    for c in range(NCORES):
        oc = np.asarray(res.results[c]["out"], np.float32)   # [128, SB]
        arr = oc.reshape(8, CCH, SB).transpose(0, 2, 1).reshape(8 * SB, CCH)
        out[c * NPC:(c + 1) * NPC] = arr[perms[c]]
    return out



# revision 27
# speedup vs baseline: 1.0143x; 1.0042x over previous
"""APPNP (K=3, alpha=0.5) on 8 Trainium2 NeuronCores.

Distribution: 1D node partition (12500 dst-nodes per core), weights
replicated. Per propagation step each core re-gathers the full h-tilde
table (AllGather) and processes the ~400k edges whose dst it owns.

Per-core per-step pipeline (edges pre-sorted by dst on host, grouped by
src-chunk into 8 GPSIMD groups x 8 node-eighths):
  ap_gather (Q7, 8-way)  : msgs[g,ch,i] = table[src]        (f32)
  tensor_tensor_scan(DVE): P = inclusive prefix sum of msgs (f32, in-place)
  ap_gather (Q7)         : EN[j] = P[last-edge-slot(node j)]
  subtract (DVE)         : per-node segment sums (diff of adjacent ends)
  matmul (PE)            : sum the 8 src-group partials into packed layout
  axpby (DVE)            : h_next = w1*agg + nh0 ; AllGather -> table
"""
import numpy as np

# ---------------------------------------------------------------- config
NCORES = 8
CCH = 16          # channels
FEAT = 512        # input features
K_STEPS = 3
ALPHA = 0.5

# real-problem geometry (overridable for small-scale tests)
N_NODES = 100000
NPC = 12500       # nodes per core
SB = 1568         # nodes per eighth (sub-block); NPC_PAD = 8*SB
WIN = SB // 4     # PE window (<=512, one PSUM bank)


def _geom():
    npc_pad = 8 * SB
    assert WIN * 4 == SB and WIN <= 512
    assert NPC <= npc_pad <= 32768 - 1
    assert SB % 32 == 0  # SBC even: int16 idx slices stay 4-byte aligned
    return npc_pad


# ---------------------------------------------------------------- host prep
def _wrap16(arr):
    """[L] -> wrapped [16, L/16] layout (element i at [i%16, i//16])."""
    L = arr.shape[-1]
    assert L % 16 == 0
    return arr.reshape(L // 16, 16).T


def _pack_nodevec(v, npc_pad):
    """per-node vector [<=NPC] -> packed [128, SB]: row 16k+ch = sub-block
    k's values (replicated over ch); padded nodes -> 0."""
    vp = np.zeros(npc_pad, np.float32)
    vp[: v.shape[0]] = v
    blocks = vp.reshape(8, SB)
    out = np.zeros((128, SB), np.float32)
    for k in range(8):
        out[16 * k:16 * k + 16, :] = blocks[k][None, :]
    return out


def _to_bf16(a):
    import ml_dtypes
    return np.asarray(a).astype(ml_dtypes.bfloat16)


def _balance_eighths(dl, gl, rng_seed=0):
    """Assign this core's dst nodes to the 8 eighths so per-(group, eighth)
    edge counts are balanced (shrinks the padded stream length NH).
    dl/gl: per-edge local dst + src group. Returns perm[node] -> position."""
    deg = np.zeros((NPC, 8), np.int64)
    np.add.at(deg, (dl, gl), 1)
    tot = deg.sum(axis=1)
    order = np.argsort(-tot, kind="stable")
    loads = np.zeros((8, 8), np.int64)
    cnt = np.zeros(8, np.int64)
    cap = np.full(8, SB, np.int64)
    cap[7] = SB - 1          # keep the last slot of eighth 7 a zero pad
    perm = np.zeros(NPC, np.int64)
    for nid in order:
        d = deg[nid]
        cand = loads + d[None, :]
        score = cand.max(axis=1)
        score[cnt >= cap] = 1 << 60
        k = int(np.argmin(score))
        loads[k] += d
        perm[nid] = k * SB + cnt[k]
        cnt[k] += 1
    return perm


def prepare_inputs(x, W, b, src, dst):
    """Build per-core in_maps + global padded stream length NH."""
    npc_pad = _geom()
    n = x.shape[0]
    assert n == NCORES * NPC

    src = np.asarray(src, dtype=np.int64)
    dst = np.asarray(dst, dtype=np.int64)
    deg_out = np.bincount(src, minlength=n).astype(np.float32)
    deg_in = np.bincount(dst, minlength=n).astype(np.float32)
    norm_out = np.clip(deg_out, 1.0, None) ** -0.5
    norm_in = np.clip(deg_in, 1.0, None) ** -0.5

    owner = dst // NPC
    group = src // NPC
    srcloc = (src - group * NPC).astype(np.int32)
    dstloc = (dst - owner * NPC).astype(np.int32)

    # balanced node -> packed-position permutation per core
    perms = []
    pos_of = np.zeros_like(dstloc)
    for c in range(NCORES):
        m = owner == c
        perm = _balance_eighths(dstloc[m], group[m])
        perms.append(perm)
        pos_of[m] = perm[dstloc[m]].astype(np.int32)
    # gather indices address the src core's table, which lives in that
    # core's permuted position space
    pos_src = np.zeros_like(srcloc)
    for g in range(NCORES):
        m = group == g
        pos_src[m] = perms[g][srcloc[m]].astype(np.int32)
    srcloc = pos_src

    cell = (owner * 8 + group).astype(np.int64)
    order = np.argsort(cell * npc_pad + pos_of, kind="stable")
    srcloc_s = srcloc[order]
    dstloc_s = pos_of[order]
    cell_s = cell[order]
    cell_start = np.searchsorted(cell_s, np.arange(65), side="left")

    # pass 1: max eighth-stream length
    nh_max = 0
    cells = {}
    for c in range(NCORES):
        for g in range(8):
            lo, hi = cell_start[c * 8 + g], cell_start[c * 8 + g + 1]
            dl = dstloc_s[lo:hi]
            sl = srcloc_s[lo:hi]
            kb = np.searchsorted(dl, np.arange(0, npc_pad + 1, SB), side="left")
            cells[(c, g)] = (dl, sl, kb)
            nh_max = max(nh_max, int(np.max(kb[1:] - kb[:-1])))
    NH = ((nh_max + 1 + 31) // 32) * 32  # +1 sentinel; %32 keeps every
    # int16 idx slice offset 4-byte aligned (Q7 reads idx by words)
    assert NH <= 32767, f"stream too long: {NH}"
    NHC = NH // 16
    SBC = SB // 16
    SENT = np.int16(npc_pad - 1)                  # table col holding 0.0
    # (eighth 7 is capped at SB-1 real nodes, so the last slot is a pad)

    wt = np.ascontiguousarray(W.T).astype(np.float32)      # [FEAT, 16]
    wtA = np.zeros((FEAT, 32), np.float32)
    wtB = np.zeros((FEAT, 32), np.float32)
    wtA[:, :16] = wt
    wtB[:, 16:] = wt
    # pre-arranged for SBUF: [128, KC*32], chunk kc at cols 32kc..
    KC = FEAT // 128
    wtA = wtA.reshape(KC, 128, 32).transpose(1, 0, 2).reshape(128, KC * 32)
    wtB = wtB.reshape(KC, 128, 32).transpose(1, 0, 2).reshape(128, KC * 32)
    g32A = np.zeros((128, 32), np.float32)
    g32B = np.zeros((128, 32), np.float32)
    for p in range(128):
        g32A[p, p % 16] = 1.0
        g32B[p, 16 + p % 16] = 1.0
    bh32 = np.tile((ALPHA * b).astype(np.float32), 2).reshape(32, 1)

    in_maps = []
    for c in range(NCORES):
        gidx = np.full((128, 8 * NHC), SENT, np.int16)
        bidx = np.zeros((128, 8 * SBC), np.int16)
        for g in range(8):
            dl, sl, kb = cells[(c, g)]
            for k in range(8):
                lo, hi = kb[k], kb[k + 1]
                stream = np.full(NH, SENT, np.int16)
                stream[1:1 + hi - lo] = sl[lo:hi].astype(np.int16)
                gidx[16 * g:16 * g + 16, k * NHC:(k + 1) * NHC] = _wrap16(stream)
                ends = np.searchsorted(
                    dl[lo:hi], np.arange(SB * k, SB * (k + 1)), side="right"
                ).astype(np.int16)
                bidx[16 * g:16 * g + 16, k * SBC:(k + 1) * SBC] = _wrap16(ends)

        rows = slice(c * NPC, (c + 1) * NPC)
        perm = perms[c]
        xt = np.zeros((FEAT, npc_pad), np.float32)
        xt[:, perm] = x[rows].T
        no_c = np.zeros(npc_pad, np.float32)
        ni_c = np.zeros(npc_pad, np.float32)
        no_c[perm] = norm_out[rows]
        ni_c[perm] = norm_in[rows]
        in_maps.append({
            "xt": _to_bf16(xt),
            "wtA": _to_bf16(wtA),
            "wtB": _to_bf16(wtB),
            "g32A": g32A,
            "g32B": g32B,
            "bh32": bh32,
            "normoutp": _pack_nodevec(no_c, npc_pad),
            "w1p": _pack_nodevec((1 - ALPHA) * ni_c * no_c, npc_pad),
            "w2p": _pack_nodevec((1 - ALPHA) * ni_c, npc_pad),
            "gidx": gidx,
            "bidx": bidx,
        })
    return in_maps, NH, perms


# ---------------------------------------------------------------- device build
def build(NH):
    import concourse.mybir as mybir
    from concourse import bacc
    import concourse.tile as tile

    F32 = mybir.dt.float32
    BF16 = mybir.dt.bfloat16
    I16 = mybir.dt.int16
    npc_pad = _geom()
    NHC = NH // 16
    SBC = SB // 16
    KC = FEAT // 128

    nc = bacc.Bacc(None, target_bir_lowering=False, num_devices=NCORES)
    xt_e = nc.declare_dram_parameter("xt", [FEAT, npc_pad], BF16, isOutput=False)
    wtA_e = nc.declare_dram_parameter("wtA", [128, (FEAT // 128) * 32], BF16,
                                      isOutput=False)
    wtB_e = nc.declare_dram_parameter("wtB", [128, (FEAT // 128) * 32], BF16,
                                      isOutput=False)
    g32A_e = nc.declare_dram_parameter("g32A", [128, 32], F32, isOutput=False)
    g32B_e = nc.declare_dram_parameter("g32B", [128, 32], F32, isOutput=False)
    bh32_e = nc.declare_dram_parameter("bh32", [32, 1], F32, isOutput=False)
    nop_e = nc.declare_dram_parameter("normoutp", [128, SB], F32, isOutput=False)
    w1p_e = nc.declare_dram_parameter("w1p", [128, SB], F32, isOutput=False)
    w2p_e = nc.declare_dram_parameter("w2p", [128, SB], F32, isOutput=False)
    gidx_e = nc.declare_dram_parameter("gidx", [128, 8 * NHC], I16, isOutput=False)
    bidx_e = nc.declare_dram_parameter("bidx", [128, 8 * SBC], I16, isOutput=False)
    out_e = nc.declare_dram_parameter("out", [128, SB], F32, isOutput=True)
    import os
    DBG = bool(int(os.environ.get("KM_DEBUG", "0")))
    if DBG:
        dbg_tab_e = nc.declare_dram_parameter("dbg_tab", [128, npc_pad], F32,
                                              isOutput=True)
        dbg_p_e = nc.declare_dram_parameter("dbg_p", [128, NH], F32,
                                            isOutput=True)
        dbg_en_e = nc.declare_dram_parameter("dbg_en", [128, SB + 4], F32,
                                             isOutput=True)
        dbg_dif_e = nc.declare_dram_parameter("dbg_dif", [128, SB], F32,
                                              isOutput=True)

    ADD = mybir.AluOpType.add
    SUB = mybir.AluOpType.subtract
    MUL = mybir.AluOpType.mult
    BYP = mybir.AluOpType.bypass
    COPY = mybir.ActivationFunctionType.Copy

    with tile.TileContext(nc) as tc:
        with (
            tc.tile_pool(name="sbuf", bufs=1) as pool,
            tc.tile_pool(name="dram", bufs=1, space="DRAM") as dram,
        ):
            # --- persistent tiles
            table = pool.tile([128, npc_pad], F32, name="table")
            gidx = pool.tile([128, 8 * NHC], I16, name="gidx")
            bidx = pool.tile([128, 8 * SBC], I16, name="bidx")
            g32A = pool.tile([128, 32], F32, name="g32A")
            g32B = pool.tile([128, 32], F32, name="g32B")
            bh32 = pool.tile([32, 1], F32, name="bh32")
            ah0p = pool.tile([128, SB], F32, name="ah0p")
            nh0p = pool.tile([128, SB], F32, name="nh0p")
            nc.sync.dma_start(out=gidx[:], in_=gidx_e[:, :])
            nc.sync.dma_start(out=bidx[:], in_=bidx_e[:, :])
            nc.sync.dma_start(out=g32A[:], in_=g32A_e[:, :])
            nc.sync.dma_start(out=g32B[:], in_=g32B_e[:, :])
            nc.sync.dma_start(out=bh32[:], in_=bh32_e[:, :])

            # --- projection: packed ah0p = alpha*h0 (h0 = x @ W.T + b)
            wtA_t = pool.tile([128, KC * 32], BF16, name="wtA_t")
            wtB_t = pool.tile([128, KC * 32], BF16, name="wtB_t")
            nc.sync.dma_start(out=wtA_t[:], in_=wtA_e[:, :])
            nc.sync.dma_start(out=wtB_t[:], in_=wtB_e[:, :])

            with tc.tile_pool(name="projpsum", bufs=1, space="PSUM") as ppsum:
                for q in range(4):
                    pp = ppsum.tile([32, 4 * 512], F32, tag="projp", bufs=2,
                                    name=f"pp{q}")
                    xtq = {}
                    for kc in range(KC):
                        xt_t = pool.tile([128, 2 * SB], BF16, tag="xt",
                                         bufs=5, name=f"xt{q}{kc}")
                        eng = nc.sync if kc % 2 == 0 else nc.scalar
                        eng.dma_start(
                            out=xt_t[:],
                            in_=xt_e[128 * kc:128 * (kc + 1),
                                     2 * q * SB:2 * (q + 1) * SB])
                        xtq[kc] = xt_t
                    for w4 in range(4):
                        first = True
                        for e in range(2):
                            wt_t = wtA_t if e == 0 else wtB_t
                            base = SB * e + WIN * w4
                            for kc in range(KC):
                                nc.tensor.matmul(
                                    out=pp[:, 512 * w4:512 * w4 + WIN],
                                    lhsT=wt_t[:, 32 * kc:32 * (kc + 1)],
                                    rhs=xtq[kc][:, base:base + WIN],
                                    start=first,
                                    stop=(e == 1 and kc == KC - 1),
                                )
                                first = False
                    # ah0p[32q..] = ALPHA*psum + ALPHA*b
                    nc.vector.tensor_scalar(
                        out=ah0p[32 * q:32 * (q + 1), :].rearrange(
                            "p (w j) -> p w j", w=4),
                        in0=pp[:, :].rearrange(
                            "p (w j) -> p w j", w=4)[:, :, :WIN],
                        scalar1=ALPHA, scalar2=bh32[:, :],
                        op0=MUL, op1=ADD)

            normoutp = pool.tile([128, SB], F32, tag="wslot", bufs=1,
                                 name="normoutp")
            nc.sync.dma_start(out=normoutp[:], in_=nop_e[:, :])
            nc.vector.tensor_tensor(out=nh0p[:], in0=ah0p[:], in1=normoutp[:],
                                    op=MUL)

            agi = dram.tile([CCH, npc_pad], F32, name="agi")
            ago = dram.tile([128, npc_pad], F32, name="ago")

            def broadcast_table(hp_f32):
                """hp (packed f32) -> AllGather -> f32 table."""
                nc.sync.dma_start(
                    out=agi[:, :].rearrange("c (k j) -> k c j", k=8),
                    in_=hp_f32[:])
                nc.gpsimd.collective_compute(
                    "AllGather", BYP,
                    replica_groups=[list(range(NCORES))],
                    ins=[agi.opt()], outs=[ago.opt()])
                half = npc_pad // 2
                nc.sync.dma_start(out=table[:, :half], in_=ago[:, :half])
                nc.scalar.dma_start(out=table[:, half:], in_=ago[:, half:])

            # h~_0 = 2 * nh0p   (= norm_out * h0 for ALPHA=0.5)
            hp0 = pool.tile([128, SB], F32, tag="hp", bufs=2, name="hp0")
            nc.vector.tensor_scalar(out=hp0[:], in0=nh0p[:], scalar1=1.0 / ALPHA,
                                    scalar2=None, op0=MUL)
            broadcast_table(hp0)
            if DBG:
                nc.sync.dma_start(out=dbg_tab_e[:, :], in_=table[:])

            w1p = pool.tile([128, SB], F32, tag="wslot", bufs=1, name="w1p")
            nc.sync.dma_start(out=w1p[:], in_=w1p_e[:, :])

            # --- propagation steps
            with tc.tile_pool(name="aggpsum", bufs=1, space="PSUM") as apsum:
                for t in range(K_STEPS):
                    last = t == K_STEPS - 1
                    if last:
                        w2p = pool.tile([128, SB], F32, tag="wslot", bufs=1,
                                        name="w2p")
                        nc.sync.dma_start(out=w2p[:], in_=w2p_e[:, :])
                    ap = apsum.tile([128, 4 * 512], F32, tag="agg", bufs=2,
                                    name=f"agg{t}")
                    hp = pool.tile([128, SB], F32, tag="hp", bufs=2,
                                   name=f"hp{t + 1}")
                    pbl = {}
                    for k in range(9):
                        if k < 8:
                            msgs = pool.tile([128, NH], F32, tag="msgs",
                                             bufs=2, name=f"m{t}_{k}")
                            nc.gpsimd.ap_gather(
                                out_ap=msgs[:], in_ap=table[:],
                                idxs_ap=gidx[:, k * NHC:(k + 1) * NHC],
                                channels=128, num_elems=npc_pad, d=1,
                                num_idxs=NH)
                            # in-place inclusive prefix sum
                            import os as _os
                            _scan_eng = nc.gpsimd if _os.environ.get(
                                "KM_SCAN_POOL") else nc.vector
                            _scan_eng.tensor_tensor_scan(
                                out=msgs[:], data0=msgs[:], data1=msgs[:],
                                initial=0.0, op0=ADD, op1=BYP)
                            pbl[k] = msgs
                        if k >= 1:
                            kk = k - 1
                            en = pool.tile([128, SB + 4], F32, tag="en",
                                           bufs=2, name=f"en{t}_{kk}")
                            nc.gpsimd.ap_gather(
                                out_ap=en[:, 1:SB + 1], in_ap=pbl[kk][:],
                                idxs_ap=bidx[:, kk * SBC:(kk + 1) * SBC],
                                channels=128, num_elems=NH, d=1, num_idxs=SB)
                            nc.vector.memset(en[:, 0:1], 0)
                            dif = pool.tile([128, SB], F32, tag="dif", bufs=2,
                                            name=f"d{t}_{kk}")
                            nc.vector.tensor_tensor(
                                out=dif[:], in0=en[:, 1:SB + 1],
                                in1=en[:, 0:SB], op=SUB)
                            if DBG and t == 0 and kk == int(os.environ.get("KM_DBG_KK", "0")):
                                nc.sync.dma_start(out=dbg_p_e[:, :],
                                                  in_=pbl[kk][:])
                                nc.sync.dma_start(out=dbg_en_e[:, :],
                                                  in_=en[:])
                                nc.sync.dma_start(out=dbg_dif_e[:, :],
                                                  in_=dif[:])
                            e, q = kk % 2, kk // 2
                            for w in range(4):
                                nc.tensor.matmul(
                                    out=ap[32 * q:32 * (q + 1),
                                           512 * w:512 * w + WIN],
                                    lhsT=(g32A if e == 0 else g32B)[:, :],
                                    rhs=dif[:, WIN * w:WIN * (w + 1)],
                                    start=(e == 0), stop=(e == 1),
                                    tile_position=(0, 32 * q))
                            # after eighth 5: rows 0:96 final -> update +
                            # AllGather chunk A early so it overlaps the
                            # remaining eighths; chunk B (rows 96:128) +
                            # table fills land at the step boundary.
                            chunk = None
                            if kk == 5:
                                chunk = (0, 96)
                            elif kk == 7:
                                chunk = (96, 128)
                            if chunk is not None:
                                r0, r1 = chunk
                                wsel = w2p if last else w1p
                                asel = ah0p if last else nh0p
                                nc.vector.tensor_tensor(
                                    out=hp[r0:r1, :].rearrange(
                                        "p (w j) -> p w j", w=4),
                                    in0=ap[r0:r1, :].rearrange(
                                        "p (w j) -> p w j", w=4)[:, :, :WIN],
                                    in1=wsel[r0:r1, :].rearrange(
                                        "p (w j) -> p w j", w=4),
                                    op=MUL)
                                nc.vector.tensor_tensor(
                                    out=hp[r0:r1, :], in0=hp[r0:r1, :],
                                    in1=asel[r0:r1, :], op=ADD)
                                if not last:
                                    nk = (r1 - r0) // 16
                                    agiq = dram.tile(
                                        [CCH, nk * SB], F32, tag=f"agi{r0}",
                                        bufs=2, name=f"agi{t}_{r0}")
                                    agoq = dram.tile(
                                        [128, nk * SB], F32, tag=f"ago{r0}",
                                        bufs=2, name=f"ago{t}_{r0}")
                                    nc.sync.dma_start(
                                        out=agiq[:, :].rearrange(
                                            "c (k j) -> k c j", k=nk),
                                        in_=hp[r0:r1, :])
                                    nc.gpsimd.collective_compute(
                                        "AllGather", BYP,
                                        replica_groups=[list(range(NCORES))],
                                        ins=[agiq.opt()], outs=[agoq.opt()])
                                    if kk == 7:
                                        # boundary fills: both chunks, two
                                        # queues, interleaved halves
                                        cA = 6 * SB
                                        nc.sync.dma_start(
                                            out=table[:, :cA // 2],
                                            in_=agoqA[:, :cA // 2])
                                        nc.scalar.dma_start(
                                            out=table[:, cA // 2:cA],
                                            in_=agoqA[:, cA // 2:])
                                        nc.sync.dma_start(
                                            out=table[:, cA:cA + SB],
                                            in_=agoq[:, :SB])
                                        nc.scalar.dma_start(
                                            out=table[:, cA + SB:],
                                            in_=agoq[:, SB:])
                                    else:
                                        agoqA = agoq
                    if last:
                        nc.sync.dma_start(out=out_e[:, :], in_=hp[:])
    nc.compile()
    return nc


# ---------------------------------------------------------------- entry point
LAST_EXEC_NS = None


def kernel(x, W, b, src, dst):
    import os
    global LAST_EXEC_NS
    x = np.asarray(x, np.float32)
    W = np.asarray(W, np.float32)
    b = np.asarray(b, np.float32)
    in_maps, NH, perms = prepare_inputs(x, W, b, src, dst)
    nc = build(NH)
    from concourse.bass_utils import run_bass_kernel_spmd
    trace = bool(int(os.environ.get("KM_TRACE", "0")))
    res = run_bass_kernel_spmd(nc, in_maps, core_ids=list(range(NCORES)),
                               trace=trace)
    LAST_EXEC_NS = res.exec_time_ns
    out = np.empty((NCORES * NPC, CCH), np.float32)
    for c in range(NCORES):
        oc = np.asarray(res.results[c]["out"], np.float32)   # [128, SB]
        arr = oc.reshape(8, CCH, SB).transpose(0, 2, 1).reshape(8 * SB, CCH)
        out[c * NPC:(c + 1) * NPC] = arr[perms[c]]
    return out

